# revision 4
# baseline (speedup 1.0000x reference)
"""Trainium2 Bass kernel v2: fused MHA block with AllGather-sharded K/V.

Sharding: 8 cores = 2 batches x 4 token-chunks of 512. Each core computes
K^T/V/Q^T projections for ONLY its 512-token chunk (tokens serve as both
its queries and its share of the keys), AllGathers K^T and V within its
4-core batch group (split into halves to pipeline), then computes full
attention for its 512 queries over all 2048 keys, output projection,
residual add and LayerNorm. This removes the 4x-replicated K/V projection
work of v1 (7.5G -> 4.3G MACs/core).

The program is identical on all cores (no core-id dependence): gathered
chunks land in natural key order on every core; only the input data
(xt/xq) differs per core.

Device-side layouts (per core):
  xt   [1024, 512]  bf16  x[b, q0:q0+512]^T (feature-major)
  xq   [512, 1024]  f32   chunk rows of x[b] (residual input)
  wq/wk/wv [1024, 1024] bf16  [c, h*64] (head-minor)
  wo   [1024, 1024] bf16  [(h*64+d), m]
  bias [16, 128]    f32   additive key bias, natural order, [jt, within]
  gamma/beta [1024] bf16
Output: y [512, 1024] f32.
"""

import contextlib

import numpy as np
import ml_dtypes

import concourse.bass as bass
import concourse.tile as tile
from concourse import mybir
from concourse import bass_utils

BF16 = ml_dtypes.bfloat16
N_CORES = 8
B, L, D, H, DH = 2, 2048, 1024, 16, 64
Q = L // 4          # tokens (queries and keys) per core
CT = D // 128       # contraction tiles over features
JT = L // 128       # key tiles (global)
IT = Q // 128       # query tiles
LN_EPS = 1e-5
GROUPS = [[0, 1, 2, 3], [4, 5, 6, 7]]

F32 = mybir.dt.float32
BF = mybir.dt.bfloat16

KT_HALF = 128 * 4 * 512        # elems per kt AG contribution (4 dt-pairs)
V_HALF = 128 * 2 * 16 * 64     # elems per v AG contribution (2 key tiles)


def _split_waits(nc, maxw=1):
    """This walrus build rejects instructions with more than one sync wait;
    split excess waits into preceding NOPs on the same engine."""
    ctr = 0
    for fn in nc.m.functions:
        for bb in fn.blocks:
            new_insts = []
            for inst in bb.instructions:
                si = inst.sync_info
                if si is not None and len(si.on_wait) > maxw:
                    waits = list(si.on_wait)
                    excess, keep = waits[:-maxw], waits[-maxw:]
                    for i in range(0, len(excess), maxw):
                        ctr += 1
                        new_insts.append(mybir.InstNoOp(
                            name=f"waitsplit_nop_{ctr}",
                            engine=inst.engine,
                            sync_info=mybir.SyncInfo(
                                on_wait=excess[i:i + maxw], on_update=[]),
                            text_hint="waitsplit",
                        ))
                    si.on_wait = keep
                new_insts.append(inst)
            bb.instructions = new_insts
    return ctr


def _bcast_dram(h1d, parts=128):
    a = h1d.ap()
    return bass.AP(tensor=a.tensor, offset=a.offset,
                   ap=[[0, parts]] + list(a.ap))


def _emit(nc, tc, hh, masked):
    Exp = mybir.ActivationFunctionType.Exp
    Sqrt = mybir.ActivationFunctionType.Sqrt

    xt_ap = hh["xt"].ap().rearrange("(t p) l -> p t l", p=128)   # [128,8,512]
    wq_ap = hh["wq"].ap().rearrange("(t p) d -> p t d", p=128)
    wk_ap = hh["wk"].ap().rearrange("(t p) d -> p t d", p=128)
    wv_ap = hh["wv"].ap().rearrange("(t p) d -> p t d", p=128)
    wo_ap = hh["wo"].ap().rearrange("(t p) d -> p t d", p=128)
    bias_ap = hh["bias"].ap().rearrange("a b -> b a")            # [128,16]
    xq_ap = hh["xq"].ap()
    y_ap = hh["y"].ap()

    ck1i = hh["cck1_in"].ap().rearrange("a (p t l) -> (a p) t l", p=128, t=4)
    ck2i = hh["cck2_in"].ap().rearrange("a (p t l) -> (a p) t l", p=128, t=4)
    cv1i = hh["ccv1_in"].ap().rearrange(
        "a (p t h d) -> (a p) t h d", p=128, t=2, h=16)
    cv2i = hh["ccv2_in"].ap().rearrange(
        "a (p t h d) -> (a p) t h d", p=128, t=2, h=16)

    def ck_out(h_, r):
        return h_.ap()[r:r + 1].rearrange(
            "a (p t l) -> (a p) t l", p=128, t=4)

    def cv_out(h_, r):
        return h_.ap()[r:r + 1].rearrange(
            "a (p t h d) -> (a p) t h d", p=128, t=2, h=16)

    with contextlib.ExitStack() as ctx:
        const = ctx.enter_context(tc.tile_pool(name="const", bufs=1))
        wpool = ctx.enter_context(tc.tile_pool(name="wpool", bufs=3))
        xtp = ctx.enter_context(tc.tile_pool(name="xtp", bufs=1))
        locp = ctx.enter_context(tc.tile_pool(name="locp", bufs=1))
        bigp = ctx.enter_context(tc.tile_pool(name="bigp", bufs=1))
        expp = ctx.enter_context(tc.tile_pool(name="expp", bufs=10))
        npool = ctx.enter_context(tc.tile_pool(name="npool", bufs=4))
        xqp = ctx.enter_context(tc.tile_pool(name="xqp", bufs=1))
        pvp = ctx.enter_context(tc.tile_pool(name="pvp", bufs=2))
        lnp = ctx.enter_context(tc.tile_pool(name="lnp", bufs=3))
        statp = ctx.enter_context(tc.tile_pool(name="statp", bufs=4))
        psS = ctx.enter_context(tc.tile_pool(name="psS", bufs=2, space="PSUM"))
        psP = ctx.enter_context(tc.tile_pool(name="psP", bufs=3, space="PSUM"))
        psD = ctx.enter_context(tc.tile_pool(name="psD", bufs=1, space="PSUM"))

        # ---- constants / small loads ----
        eps_sb = const.tile([128, 1], F32)
        nc.vector.memset(eps_sb[:], LN_EPS)
        ones64 = const.tile([1, 64], BF)   # den-broadcast lhsT
        nc.vector.memset(ones64[:], 1.0)
        bias_sb = const.tile([128, 16], F32)
        nc.gpsimd.dma_start(out=bias_sb[:], in_=bias_ap)
        gamma_sb = const.tile([128, 1024], BF)
        beta_sb = const.tile([128, 1024], BF)

        # ---- big SBUF tensors ----
        kt_all = bigp.tile([128, 8, 2048], BF)     # K^T by dt-pair, all keys
        v_all = bigp.tile([128, JT, H, DH + 1], BF)
        qt_all = bigp.tile([128, 8, Q], BF)        # Q^T for the chunk
        probt = bigp.tile([128, 8, Q], BF)         # normalized P^T stacked
        kt_loc = locp.tile([128, 8, Q], BF)        # local K^T (pre-gather)
        v_loc = locp.tile([128, 4, H, DH], BF)     # local V (pre-gather)

        nc.vector.memset(v_all[:, :, :, DH:DH + 1], 1.0)

        # ---- input loads (split across queues for early start) ----
        xt_sb = xtp.tile([128, 8, 512], BF, tag="xt")
        nc.sync.dma_start(out=xt_sb[:, 0:4, :], in_=xt_ap[:, 0:4, :])
        nc.scalar.dma_start(out=xt_sb[:, 4:8, :], in_=xt_ap[:, 4:8, :])
        wk_sb = wpool.tile([128, 8, 1024], BF, tag="w")
        nc.sync.dma_start(out=wk_sb[:, 0:4, :], in_=wk_ap[:, 0:4, :])
        nc.scalar.dma_start(out=wk_sb[:, 4:8, :], in_=wk_ap[:, 4:8, :])
        wq_sb = wpool.tile([128, 8, 1024], BF, tag="w")
        nc.sync.dma_start(out=wq_sb[:], in_=wq_ap)
        wv_sb = wpool.tile([128, 8, 1024], BF, tag="w")
        nc.scalar.dma_start(out=wv_sb[:], in_=wv_ap)

        def proj_pair(w_sb, dtp, dst):
            """Two feature-column tiles (dt=2dtp, 2dtp+1) of a [D,D] proj
            for the local 512 tokens -> dst [128, 2, 512] (transposed)."""
            ps = psS.tile([128, 2, 512], F32, tag="ss")
            for half in range(2):
                dt = 2 * dtp + half
                for ct in range(CT):
                    nc.tensor.matmul(
                        ps[:, half, :],
                        w_sb[:, ct, dt * 128:(dt + 1) * 128],
                        xt_sb[:, ct, :],
                        start=(ct == 0), stop=(ct == CT - 1))
            nc.vector.tensor_copy(dst, ps[:])

        # ---- K^T proj first half (dt 0-3) + AG ----
        for dtp in range(2):
            proj_pair(wk_sb, dtp, kt_loc[:, 2 * dtp:2 * dtp + 2, :])
        nc.gpsimd.dma_start(out=ck1i, in_=kt_loc[:, 0:4, :])
        nc.gpsimd.collective_compute(
            "AllGather", mybir.AluOpType.bypass, replica_groups=GROUPS,
            ins=[hh["cck1_in"].ap()], outs=[hh["cck1_out"].ap()])
        for r in range(4):
            eng = nc.sync if r % 2 == 0 else nc.scalar
            eng.dma_start(out=kt_all[:, 0:4, r * 512:(r + 1) * 512],
                          in_=ck_out(hh["cck1_out"], r))

        # ---- Q^T proj first pair (dt 0-1) so attention dt0 can start ----
        proj_pair(wq_sb, 0, qt_all[:, 0:2, :])

        # ---- V proj halves + AGs ----
        def v_proj(lt):
            ps = psS.tile([128, 2, 512], F32, tag="ss")
            for nt in range(2):
                for ct in range(CT):
                    nc.tensor.matmul(
                        ps[:, nt, :],
                        xt_sb[:, ct, lt * 128:(lt + 1) * 128],
                        wv_sb[:, ct, nt * 512:(nt + 1) * 512],
                        start=(ct == 0), stop=(ct == CT - 1))
            nc.vector.tensor_copy(
                v_loc[:, lt, :, :],
                ps.rearrange("p n (h d) -> p (n h) d", h=8))

        for lt in range(2):
            v_proj(lt)
        nc.gpsimd.dma_start(out=cv1i, in_=v_loc[:, 0:2, :, :])
        nc.gpsimd.collective_compute(
            "AllGather", mybir.AluOpType.bypass, replica_groups=GROUPS,
            ins=[hh["ccv1_in"].ap()], outs=[hh["ccv1_out"].ap()])
        for r in range(4):
            eng = nc.sync if r % 2 == 0 else nc.scalar
            eng.dma_start(out=v_all[:, 4 * r:4 * r + 2, :, 0:DH],
                          in_=cv_out(hh["ccv1_out"], r))

        # ---- Q^T proj rest (dt 2-7) ----
        for dtp in range(1, 4):
            proj_pair(wq_sb, dtp, qt_all[:, 2 * dtp:2 * dtp + 2, :])

        for lt in range(2, 4):
            v_proj(lt)
        nc.gpsimd.dma_start(out=cv2i, in_=v_loc[:, 2:4, :, :])
        nc.gpsimd.collective_compute(
            "AllGather", mybir.AluOpType.bypass, replica_groups=GROUPS,
            ins=[hh["ccv2_in"].ap()], outs=[hh["ccv2_out"].ap()])
        for r in range(4):
            eng = nc.sync if r % 2 == 0 else nc.scalar
            eng.dma_start(out=v_all[:, 4 * r + 2:4 * r + 4, :, 0:DH],
                          in_=cv_out(hh["ccv2_out"], r))

        # ---- K^T proj second half (dt 4-7) + AG ----
        for dtp in range(2, 4):
            proj_pair(wk_sb, dtp, kt_loc[:, 2 * dtp:2 * dtp + 2, :])
        nc.gpsimd.dma_start(out=ck2i, in_=kt_loc[:, 4:8, :])
        nc.gpsimd.collective_compute(
            "AllGather", mybir.AluOpType.bypass, replica_groups=GROUPS,
            ins=[hh["cck2_in"].ap()], outs=[hh["cck2_out"].ap()])
        for r in range(4):
            eng = nc.sync if r % 2 == 0 else nc.scalar
            eng.dma_start(out=kt_all[:, 4:8, r * 512:(r + 1) * 512],
                          in_=ck_out(hh["cck2_out"], r))

        # wo load (reuses a wpool slot once wk is dead)
        wo_sb = wpool.tile([128, 8, 1024], BF, tag="w")
        nc.scalar.dma_start(out=wo_sb[:], in_=wo_ap)

        # ---- attention per dt (head pair) ----
        for dt in range(8):
            pv = [psP.tile([DH + 1, 512], F32, tag="pp", name=f"pv{dt}_{hb}")
                  for hb in range(2)]
            for jt in range(JT):
                ps = psS.tile([128, 2, 512], F32, tag="ss")
                for hb in range(2):
                    nc.tensor.matmul(
                        ps[:, hb, :],
                        kt_all[hb * 64:hb * 64 + 64, dt,
                               jt * 128:(jt + 1) * 128],
                        qt_all[hb * 64:hb * 64 + 64, dt, :],
                        start=True, stop=True)
                expt = expp.tile([128, 2, 512], BF, tag="e")
                if masked:
                    for hb in range(2):
                        nc.scalar.activation(
                            expt[:, hb, :], ps[:, hb, :], Exp,
                            bias=bias_sb[:, jt:jt + 1], scale=1.0 / 8.0)
                else:
                    nc.scalar.activation(
                        expt[:], ps[:], Exp, bias=0.0, scale=1.0 / 8.0)
                for hb in range(2):
                    nc.tensor.matmul(
                        pv[hb][:], v_all[:, jt, 2 * dt + hb, 0:DH + 1],
                        expt[:, hb, :],
                        start=(jt == 0), stop=(jt == JT - 1))
            # normalize: reciprocal of each tiny [1,512] denominator FIRST,
            # then broadcast it across 64 partitions via a K=1 matmul (both
            # heads into one psD bank). The PV numerator is evicted to SBUF
            # (bf16) so the final multiply reads only one PSUM operand and
            # the psP bank frees early.
            ps_d = psD.tile([128, 512], F32, tag="dd")
            pvn = []
            for hb in range(2):
                den = npool.tile([1, 512], F32, tag="n")
                nc.vector.tensor_copy(den[:], pv[hb][DH:DH + 1, :])
                rdiv = npool.tile([1, 512], BF, tag="r")
                with nc.allow_low_precision(
                        reason="bf16 1/den; probt is bf16 anyway"):
                    nc.vector.reciprocal(rdiv[:], den[:])
                nc.tensor.matmul(ps_d[hb * 64:(hb + 1) * 64, :], ones64[:],
                                 rdiv[:], start=True, stop=True)
                t = pvp.tile([64, 512], BF, tag="pv")
                nc.vector.tensor_copy(t[:], pv[hb][0:DH, :])
                pvn.append(t)
            for hb in range(2):
                nc.vector.tensor_mul(
                    probt[hb * 64:hb * 64 + 64, dt, :],
                    pvn[hb][:], ps_d[hb * 64:hb * 64 + 64, :])

        # ---- output projection + residual + LayerNorm ----
        nc.gpsimd.dma_start(out=gamma_sb[:], in_=_bcast_dram(hh["gamma"]))
        nc.gpsimd.dma_start(out=beta_sb[:], in_=_bcast_dram(hh["beta"]))
        for it in range(IT):
            xq_t = xqp.tile([128, 1024], F32, tag="xq")
            nc.sync.dma_start(out=xq_t[:],
                              in_=xq_ap[it * 128:(it + 1) * 128, :])
            ps_r = psS.tile([128, 2, 512], F32, tag="ss")
            for mh in range(2):
                for kt in range(8):
                    nc.tensor.matmul(
                        ps_r[:, mh, :],
                        probt[:, kt, it * 128:(it + 1) * 128],
                        wo_sb[:, kt, mh * 512:(mh + 1) * 512],
                        start=(kt == 0), stop=(kt == 7))
            h_sb = lnp.tile([128, 1024], F32, tag="ln")
            nc.vector.tensor_add(h_sb[:], ps_r.rearrange("p a b -> p (a b)"),
                                 xq_t[:])
            stats = statp.tile([128, 2, 6], F32)
            nc.vector.bn_stats(stats[:, 0, :], h_sb[:, 0:512])
            nc.vector.bn_stats(stats[:, 1, :], h_sb[:, 512:1024])
            mv = statp.tile([128, 2], F32)
            nc.vector.bn_aggr(mv[:], stats[:])
            std = statp.tile([128, 1], F32)
            nc.scalar.activation(std[:], mv[:, 1:2], Sqrt,
                                 bias=eps_sb[:], scale=1.0)
            rstd = statp.tile([128, 1], F32)
            nc.vector.reciprocal(rstd[:], std[:])
            t1 = lnp.tile([128, 1024], F32, tag="ln")
            nc.vector.tensor_scalar(
                t1[:], h_sb[:], mv[:, 0:1], rstd[:],
                op0=mybir.AluOpType.subtract, op1=mybir.AluOpType.mult)
            t2 = lnp.tile([128, 1024], F32, tag="ln")
            nc.vector.tensor_mul(t2[:], t1[:], gamma_sb[:])
            out_t = lnp.tile([128, 1024], F32, tag="ln")
            nc.vector.tensor_add(out_t[:], t2[:], beta_sb[:])
            nc.sync.dma_start(y_ap[it * 128:(it + 1) * 128, :], out_t[:])


def build_module(split=True, masked=False):
    nc = bass.Bass("TRN2", target_bir_lowering=False, debug=False,
                   num_devices=N_CORES)
    hh = {
        "xt": nc.dram_tensor("xt", [D, Q], BF, kind="ExternalInput"),
        "xq": nc.dram_tensor("xq", [Q, D], F32, kind="ExternalInput"),
        "wq": nc.dram_tensor("wq", [D, D], BF, kind="ExternalInput"),
        "wk": nc.dram_tensor("wk", [D, D], BF, kind="ExternalInput"),
        "wv": nc.dram_tensor("wv", [D, D], BF, kind="ExternalInput"),
        "wo": nc.dram_tensor("wo", [D, D], BF, kind="ExternalInput"),
        "bias": nc.dram_tensor("bias", [16, 128], F32, kind="ExternalInput"),
        "gamma": nc.dram_tensor("gamma", [D], BF, kind="ExternalInput"),
        "beta": nc.dram_tensor("beta", [D], BF, kind="ExternalInput"),
        "y": nc.dram_tensor("y", [Q, D], F32, kind="ExternalOutput"),
        "cck1_in": nc.dram_tensor("cck1_in", [1, KT_HALF], BF),
        "cck1_out": nc.dram_tensor("cck1_out", [4, KT_HALF], BF),
        "cck2_in": nc.dram_tensor("cck2_in", [1, KT_HALF], BF),
        "cck2_out": nc.dram_tensor("cck2_out", [4, KT_HALF], BF),
        "ccv1_in": nc.dram_tensor("ccv1_in", [1, V_HALF], BF),
        "ccv1_out": nc.dram_tensor("ccv1_out", [4, V_HALF], BF),
        "ccv2_in": nc.dram_tensor("ccv2_in", [1, V_HALF], BF),
        "ccv2_out": nc.dram_tensor("ccv2_out", [4, V_HALF], BF),
    }
    with tile.TileContext(nc) as tc:
        _emit(nc, tc, hh, masked)
    if split:
        _split_waits(nc, 1)
    return nc


_CACHE = {}


def get_module(masked=False):
    key = ("nc", masked)
    if key not in _CACHE:
        _CACHE[key] = build_module(masked=masked)
    return _CACHE[key]


def prep_inputs(x, mask, w_q, w_k, w_v, w_o, ln_gamma, ln_beta):
    x = np.asarray(x, dtype=np.float32)
    mask = np.asarray(mask)
    shared = {
        "wq": np.ascontiguousarray(
            np.asarray(w_q, np.float32).transpose(1, 0, 2).reshape(D, D)
        ).astype(BF16),
        "wk": np.ascontiguousarray(
            np.asarray(w_k, np.float32).transpose(1, 0, 2).reshape(D, D)
        ).astype(BF16),
        "wv": np.ascontiguousarray(
            np.asarray(w_v, np.float32).transpose(1, 0, 2).reshape(D, D)
        ).astype(BF16),
        "wo": np.asarray(w_o, np.float32).reshape(D, D).astype(BF16),
        "gamma": np.asarray(ln_gamma, np.float32).astype(BF16),
        "beta": np.asarray(ln_beta, np.float32).astype(BF16),
    }
    in_maps = []
    for c in range(N_CORES):
        b, q0 = c // 4, (c % 4) * Q
        m = {
            "xt": np.ascontiguousarray(x[b, q0:q0 + Q, :].T).astype(BF16),
            "xq": np.ascontiguousarray(x[b, q0:q0 + Q, :]),
            "bias": np.where(mask[b], 0.0, -1e9).astype(
                np.float32).reshape(16, 128),
        }
        m.update(shared)
        in_maps.append(m)
    masked = not bool(mask.all())
    return in_maps, masked


def assemble(results):
    out = np.empty((B, L, D), dtype=np.float32)
    for c in range(N_CORES):
        b, q0 = c // 4, (c % 4) * Q
        out[b, q0:q0 + Q, :] = results[c]["y"]
    return out


def run(in_maps, masked=False, **kwargs):
    nc = get_module(masked)
    return bass_utils.run_bass_kernel_spmd(
        nc, in_maps, core_ids=list(range(N_CORES)), **kwargs)


def kernel(x, mask, w_q, w_k, w_v, w_o, ln_gamma, ln_beta):
    in_maps, masked = prep_inputs(x, mask, w_q, w_k, w_v, w_o,
                                  ln_gamma, ln_beta)
    res = run(in_maps, masked)
    return assemble(res.results)
